# revision 1
# baseline (speedup 1.0000x reference)
"""LinearAttention Trainium2 Bass kernel.

kernel(**inputs) takes the full unsharded inputs from setup_inputs() and
returns the full output. Shards data-parallel over batch (b=8) across 8
NeuronCores; each core computes one batch item:

  qkv = w_qkv @ x            (layout B on chip: [n, 768], n on partitions)
  q = softmax_d(q); k = softmax_n(k)
  ctx[h] = ek_h^T @ v_h      (accumulated over n in PSUM; an appended ones
                              column yields sum_n ek for the k softmax)
  M^T = blockdiag(ctx/s_k)^T @ w_out^T   (folded once between passes)
  out = M @ eqnA + b_out     (eqnA = PE-transposed normalized exp(q))

Matmul operands are bf16; accumulation stays fp32 in PSUM.
"""

import numpy as np
import ml_dtypes

import concourse.bass as bass
import concourse.tile as tile
from concourse import bacc, mybir
from concourse.bass_utils import run_bass_kernel_spmd
from concourse.masks import make_identity

F32 = mybir.dt.float32
BF16 = mybir.dt.bfloat16
AF = mybir.ActivationFunctionType

C = 128
N = 16384
HEADS = 4
DH = 64
INNER = HEADS * DH          # 256
QKV = 3 * INNER             # 768
NB = 512
SUB = NB // 128
NBLK = N // NB              # 32
NSUB = N // 128             # 128


def build_nc():
    nc = bacc.Bacc("TRN2", target_bir_lowering=False, debug=False, num_devices=8)

    x = nc.dram_tensor("x", [C, N], BF16, kind="ExternalInput")
    wqT = nc.dram_tensor("wqT", [C, QKV], BF16, kind="ExternalInput")
    woT = nc.dram_tensor("woT", [INNER, C], BF16, kind="ExternalInput")
    bo = nc.dram_tensor("bo", [C, 1], F32, kind="ExternalInput")
    out = nc.dram_tensor("out", [C, N], F32, kind="ExternalOutput")

    with tile.TileContext(nc) as tc:
        with (
            tc.tile_pool(name="consts", bufs=1) as consts,
            tc.tile_pool(name="eqa", bufs=1) as eqa,
            tc.tile_pool(name="xin", bufs=4) as xin,
            tc.tile_pool(name="work", bufs=4) as work,
            tc.tile_pool(name="small", bufs=4) as small,
        ):
            wq_s = consts.tile([C, QKV], BF16)
            nc.sync.dma_start(out=wq_s, in_=wqT[:, :])
            wo_s = consts.tile([C, 2, C], BF16)
            nc.sync.dma_start(out=wo_s[:, 0, :], in_=woT[0:128, :])
            nc.sync.dma_start(out=wo_s[:, 1, :], in_=woT[128:256, :])
            bo_s = consts.tile([C, 1], F32)
            nc.sync.dma_start(out=bo_s, in_=bo[:, :])
            ident = consts.tile([C, C], BF16)
            make_identity(nc, ident)

            # layout-A normalized exp(q): [:, 0, :] = heads 0/1, [:, 1, :] = 2/3
            eqnA = eqa.tile([C, 2, N], BF16)
            MT01 = consts.tile([C, C], BF16)
            MT23 = consts.tile([C, C], BF16)

            with (
                tc.tile_pool(name="qkvp", bufs=2, space="PSUM") as qkvp,
                tc.tile_pool(name="trp", bufs=2, space="PSUM") as trp,
                tc.tile_pool(name="ctxp", bufs=1, space="PSUM") as ctxp,
            ):
                ctx01 = ctxp.tile([C, INNER + 1], F32)
                ctx23 = ctxp.tile([C, INNER + 1], F32)

                x_blk = None
                for t in range(NSUB):
                    blk, s = divmod(t, SUB)
                    if s == 0:
                        x_blk = xin.tile([C, NB], BF16, tag="x_blk")
                        nc.sync.dma_start(
                            out=x_blk, in_=x[:, blk * NB : (blk + 1) * NB]
                        )
                    xs = x_blk[:, s * 128 : (s + 1) * 128]

                    qkv = qkvp.tile([C, QKV], F32, tag="qkv")
                    nc.tensor.matmul(
                        qkv[:, 0:512], lhsT=xs, rhs=wq_s[:, 0:512],
                        start=True, stop=True, skip_group_check=True,
                    )
                    nc.tensor.matmul(
                        qkv[:, 512:768], lhsT=xs, rhs=wq_s[:, 512:768],
                        start=True, stop=True, skip_group_check=True,
                    )

                    # one exp over q|k halves; heads 0..3 = q, 4..7 = k
                    eqk = work.tile([C, 8, DH], BF16, tag="eqk")
                    nc.scalar.activation(eqk[:, :, :], qkv[:, 0:512], AF.Exp)

                    sq = small.tile([C, HEADS, 1], F32, tag="sq")
                    nc.vector.reduce_sum(
                        sq, eqk[:, 0:4, :], axis=mybir.AxisListType.X
                    )
                    rq = small.tile([C, HEADS, 1], F32, tag="rq")
                    nc.vector.reciprocal(rq, sq)
                    eqn = work.tile([C, HEADS, DH], BF16, tag="eqn")
                    nc.gpsimd.tensor_mul(
                        eqn, eqk[:, 0:4, :], rq.broadcast_to([C, HEADS, DH])
                    )

                    vt = work.tile([C, INNER + 1], BF16, tag="vt")
                    if t % 2 == 0:
                        nc.vector.tensor_copy(vt[:, 0:256], qkv[:, 512:768])
                    else:
                        nc.scalar.copy(vt[:, 0:256], qkv[:, 512:768])
                    nc.gpsimd.memset(vt[:, 256:257], 1.0)

                    nc.tensor.matmul(
                        ctx01, lhsT=eqk[:, 4:6, :], rhs=vt,
                        start=(t == 0), stop=(t == NSUB - 1), skip_group_check=True,
                    )
                    nc.tensor.matmul(
                        ctx23, lhsT=eqk[:, 6:8, :], rhs=vt,
                        start=(t == 0), stop=(t == NSUB - 1), skip_group_check=True,
                    )

                    tr = trp.tile([C, 2, C], BF16, tag="tr")
                    nc.tensor.transpose(tr[:, 0, :], eqn[:, 0:2, :], ident)
                    nc.tensor.transpose(tr[:, 1, :], eqn[:, 2:4, :], ident)
                    if t % 2 == 0:
                        nc.scalar.copy(eqnA[:, :, t * 128 : (t + 1) * 128], tr)
                    else:
                        nc.vector.tensor_copy(
                            eqnA[:, :, t * 128 : (t + 1) * 128], tr
                        )

                # ---- fold: MT = (blockdiag(ctx/s_k))^T @ w_out^T ----
                r01 = small.tile([C, 1], F32, tag="r01")
                r23 = small.tile([C, 1], F32, tag="r23")
                nc.vector.reciprocal(r01, ctx01[:, 256:257])
                nc.vector.reciprocal(r23, ctx23[:, 256:257])
                bd01 = consts.tile([C, C], BF16)
                bd23 = consts.tile([C, C], BF16)
                nc.vector.tensor_scalar_mul(
                    bd01[0:64, 0:64], ctx01[0:64, 0:64], r01[0:64, 0:1]
                )
                nc.vector.tensor_scalar_mul(
                    bd01[64:128, 64:128], ctx01[64:128, 64:128], r01[64:128, 0:1]
                )
                nc.vector.tensor_scalar_mul(bd01[0:64, 64:128], ctx01[0:64, 64:128], 0.0)
                nc.vector.tensor_scalar_mul(bd01[64:128, 0:64], ctx01[64:128, 0:64], 0.0)
                nc.vector.tensor_scalar_mul(
                    bd23[0:64, 0:64], ctx23[0:64, 128:192], r23[0:64, 0:1]
                )
                nc.vector.tensor_scalar_mul(
                    bd23[64:128, 64:128], ctx23[64:128, 192:256], r23[64:128, 0:1]
                )
                nc.vector.tensor_scalar_mul(bd23[0:64, 64:128], ctx23[0:64, 0:64], 0.0)
                nc.vector.tensor_scalar_mul(bd23[64:128, 0:64], ctx23[64:128, 0:64], 0.0)

                for pair, bd, mt in ((0, bd01, MT01), (1, bd23, MT23)):
                    tb = trp.tile([C, 2, C], BF16, tag="tr")
                    nc.tensor.transpose(tb[:, 0, :], bd, ident)
                    bdt = consts.tile([C, C], BF16, tag=f"bdt{pair}")
                    nc.vector.tensor_copy(bdt, tb[:, 0, :])
                    mtp = qkvp.tile([C, QKV], F32, tag="qkv")
                    nc.tensor.matmul(
                        mtp[:, 0:128], lhsT=bdt, rhs=wo_s[:, pair, :],
                        start=True, stop=True, skip_group_check=True,
                    )
                    nc.vector.tensor_copy(mt, mtp[:, 0:128])

            # ---- pass 2: out = MT^T @ eqnA + b ----
            with tc.tile_pool(name="finp", bufs=2, space="PSUM") as finp:
                for blk in range(NBLK):
                    nsl = slice(blk * NB, (blk + 1) * NB)
                    fin = finp.tile([C, NB], F32, tag="fin")
                    nc.tensor.matmul(
                        fin, lhsT=MT01, rhs=eqnA[:, 0, nsl],
                        start=True, stop=False, skip_group_check=True,
                    )
                    nc.tensor.matmul(
                        fin, lhsT=MT23, rhs=eqnA[:, 1, nsl],
                        start=False, stop=True, skip_group_check=True,
                    )
                    osb = work.tile([C, NB], F32, tag="osb")
                    if blk % 2 == 0:
                        nc.scalar.activation(
                            osb, fin, AF.Identity, bias=bo_s[:, 0:1], scale=1.0
                        )
                    else:
                        nc.vector.tensor_scalar_add(osb, fin, bo_s[:, 0:1])
                    nc.sync.dma_start(out=out[:, nsl], in_=osb)

    nc.compile()
    return nc


_NC_CACHE = None


def kernel(x, w_qkv, w_out, b_out):
    global _NC_CACHE
    if _NC_CACHE is None:
        _NC_CACHE = build_nc()
    nc = _NC_CACHE

    b = x.shape[0]
    bf = ml_dtypes.bfloat16
    wqT = np.ascontiguousarray(np.asarray(w_qkv, dtype=np.float32).T.astype(bf))
    woT = np.ascontiguousarray(np.asarray(w_out, dtype=np.float32).T.astype(bf))
    bo = np.ascontiguousarray(np.asarray(b_out, dtype=np.float32).reshape(C, 1))
    xb = np.asarray(x, dtype=np.float32).reshape(b, C, N).astype(bf)
    in_maps = [
        {"x": np.ascontiguousarray(xb[i]), "wqT": wqT, "woT": woT, "bo": bo}
        for i in range(b)
    ]
    res = run_bass_kernel_spmd(nc, in_maps, core_ids=list(range(b)))
    return np.stack(
        [res.results[i]["out"].reshape(C, 128, 128) for i in range(b)]
    ).astype(np.float32)



# revision 3
# speedup vs baseline: 1.4955x; 1.4955x over previous
"""LinearAttention Trainium2 Bass kernel (v2).

kernel(**inputs) takes the full unsharded inputs from setup_inputs() and
returns the full output. Shards data-parallel over batch (b=8) across 8
NeuronCores; each core computes one batch item.

Per core, with n = h*w = 16384 pixels on SBUF partitions (128 px/tile):

  qk   = x^T @ w_qk           (512-col matmul per tile, bf16 PSUM)
  eqk  = exp(qk)              (one ACT instr per 4 tiles)
  sq   = sum_d exp(q)         (DVE reduce, per head)
  eqn  = exp(q) / sq          (GpSimd broadcast multiply by 1/sq)
  G_p  = sum_n ek_p [x^T | 1] (PE accumulation in PSUM over all tiles;
                               replaces materializing v: ctx = w_v @ G^T)
  eqnA = eqn^T                (PE transposes, staged to SBUF for pass 2)

Fold (once): ctx^T = w_v^T @ (G/s_k)^T ; MT_p = blockdiag(ctx)^T @ w_out^T
Pass 2: out = MT01^T @ eqnA01 + MT23^T @ eqnA23 + b_out   (bf16 out,
upcast to fp32 on host).
"""

import numpy as np
import ml_dtypes

import concourse.bass as bass
import concourse.tile as tile
from concourse import bacc, mybir
from concourse.bass_utils import run_bass_kernel_spmd
from concourse.masks import make_identity

F32 = mybir.dt.float32
BF16 = mybir.dt.bfloat16
AF = mybir.ActivationFunctionType

C = 128
N = 16384
HEADS = 4
DH = 64
INNER = HEADS * DH          # 256
QKV = 3 * INNER             # 768
NSUB = N // 128             # 128 tiles of 128 px
GS = 4                      # sub-tiles per group (batched elementwise)
NG = NSUB // GS             # 32 groups
DMABLK = 2048               # px per DMA transfer
NBLK = N // DMABLK          # 8 DMA blocks
NB = 512                    # pass-2 block (px)


def build_nc():
    nc = bacc.Bacc("TRN2", target_bir_lowering=False, debug=False, num_devices=8)

    x = nc.dram_tensor("x", [C, N], BF16, kind="ExternalInput")
    xt = nc.dram_tensor("xt", [C, NSUB, 129], BF16, kind="ExternalInput")
    wqT = nc.dram_tensor("wqT", [C, QKV], BF16, kind="ExternalInput")
    woT = nc.dram_tensor("woT", [INNER, C], BF16, kind="ExternalInput")
    bo = nc.dram_tensor("bo", [C, 1], F32, kind="ExternalInput")
    out = nc.dram_tensor("out", [C, N], BF16, kind="ExternalOutput")

    with tile.TileContext(nc) as tc:
        with (
            tc.tile_pool(name="consts", bufs=1) as consts,
            tc.tile_pool(name="eqa", bufs=1) as eqa,
            tc.tile_pool(name="xin", bufs=2) as xin,
            tc.tile_pool(name="xtin", bufs=2) as xtin,
            tc.tile_pool(name="work", bufs=2) as work,
            tc.tile_pool(name="small", bufs=2) as small,
            tc.tile_pool(name="folds", bufs=1) as folds,
            tc.tile_pool(name="osbp", bufs=2) as osbp,
        ):
            wq_s = consts.tile([C, QKV], BF16)
            nc.sync.dma_start(out=wq_s, in_=wqT[:, :])
            wo_s = consts.tile([C, 2, C], BF16)
            nc.sync.dma_start(out=wo_s[:, 0, :], in_=woT[0:128, :])
            nc.sync.dma_start(out=wo_s[:, 1, :], in_=woT[128:256, :])
            bo_s = consts.tile([C, 1], F32)
            nc.sync.dma_start(out=bo_s, in_=bo[:, :])
            ident = consts.tile([C, C], BF16)
            make_identity(nc, ident)

            # transposed, normalized exp(q): [:, 0, :] = heads 0/1, [:, 1, :] = 2/3
            eqnA = eqa.tile([C, 2, N], BF16)
            MT01 = consts.tile([C, C], BF16)
            MT23 = consts.tile([C, C], BF16)

            with tc.tile_pool(name="gp", bufs=1, space="PSUM") as gp:
                G01 = gp.tile([C, 129], F32)
                G23 = gp.tile([C, 129], F32)

                with (
                    tc.tile_pool(name="qkp", bufs=2, space="PSUM") as qkp,
                    tc.tile_pool(name="trp", bufs=2, space="PSUM") as trp,
                ):
                    x_blk = None
                    xt_blk = None
                    for g in range(NG):
                        if g % (DMABLK // (GS * 128)) == 0:
                            blk = g // (DMABLK // (GS * 128))
                            x_blk = xin.tile([C, DMABLK], BF16, tag="x_blk")
                            nc.sync.dma_start(
                                out=x_blk,
                                in_=x[:, blk * DMABLK : (blk + 1) * DMABLK],
                            )
                            xt_blk = xtin.tile([C, 16, 129], BF16, tag="xt_blk")
                            nc.sync.dma_start(
                                out=xt_blk, in_=xt[:, blk * 16 : (blk + 1) * 16, :]
                            )

                        # exp over q|k: heads 0..3 = q, 4..7 = k
                        eqk = work.tile([C, GS, 8, DH], BF16, tag="eqk")
                        for h in range(GS // 2):
                            qk = qkp.tile([C, 2, 8, DH], F32, tag="qk")
                            for s2 in range(2):
                                s = h * 2 + s2
                                ti = (g * GS + s) % 16
                                xs = x_blk[:, ti * 128 : (ti + 1) * 128]
                                nc.tensor.matmul(
                                    qk[:, s2, :, :], lhsT=xs, rhs=wq_s[:, 0:512],
                                    start=True, stop=True, skip_group_check=True,
                                )
                            nc.scalar.activation(
                                eqk[:, 2 * h : 2 * h + 2, :, :], qk, AF.Exp
                            )

                        sq = small.tile([C, GS, HEADS, 1], F32, tag="sq")
                        nc.vector.reduce_sum(
                            sq, eqk[:, :, 0:4, :], axis=mybir.AxisListType.X
                        )
                        rq = small.tile([C, GS, HEADS, 1], F32, tag="rq")
                        nc.vector.reciprocal(rq, sq)
                        eqn = work.tile([C, GS, HEADS, DH], BF16, tag="eqn")
                        nc.gpsimd.tensor_mul(
                            eqn, eqk[:, :, 0:4, :],
                            rq.broadcast_to([C, GS, HEADS, DH]),
                        )

                        tr2 = trp.tile([C, 2, GS * 128], BF16, tag="tr2")
                        for s in range(GS):
                            t = g * GS + s
                            ti = t % 16
                            nc.tensor.matmul(
                                G01, lhsT=eqk[:, s, 4:6, :],
                                rhs=xt_blk[:, ti, :],
                                start=(t == 0), stop=(t == NSUB - 1),
                                skip_group_check=True,
                            )
                            nc.tensor.matmul(
                                G23, lhsT=eqk[:, s, 6:8, :],
                                rhs=xt_blk[:, ti, :],
                                start=(t == 0), stop=(t == NSUB - 1),
                                skip_group_check=True,
                            )
                            nc.tensor.transpose(
                                tr2[:, 0, s * 128 : (s + 1) * 128],
                                eqn[:, s, 0:2, :], ident,
                            )
                            nc.tensor.transpose(
                                tr2[:, 1, s * 128 : (s + 1) * 128],
                                eqn[:, s, 2:4, :], ident,
                            )
                        nc.vector.tensor_copy(
                            eqnA[:, :, g * (GS * 128) : (g + 1) * (GS * 128)], tr2
                        )

                # ---- fold: MT_p = blockdiag(ctx_p / s_k)^T @ w_out_p^T ----
                with tc.tile_pool(name="foldp", bufs=1, space="PSUM") as foldp:
                    for p, Gp, MTp in ((0, G01, MT01), (1, G23, MT23)):
                        rv = folds.tile([C, 1], F32, tag=f"rv{p}")
                        nc.vector.reciprocal(rv, Gp[:, 128:129])
                        Gn = folds.tile([C, C], BF16, tag=f"gn{p}")
                        nc.vector.tensor_scalar_mul(Gn, Gp[:, 0:128], rv[:, 0:1])
                        GTps = foldp.tile([C, C], BF16, tag=f"gt{p}")
                        nc.tensor.transpose(GTps, Gn, ident)
                        GTs = folds.tile([C, C], BF16, tag=f"gts{p}")
                        nc.vector.tensor_copy(GTs, GTps)
                        ctxT = foldp.tile([C, C], F32, tag=f"ctx{p}")
                        nc.tensor.matmul(
                            ctxT, lhsT=wq_s[:, 512 + 128 * p : 512 + 128 * (p + 1)],
                            rhs=GTs, start=True, stop=True, skip_group_check=True,
                        )
                        ctxs = folds.tile([C, 64], BF16, tag=f"cs{p}")
                        nc.vector.tensor_copy(ctxs[0:64, :], ctxT[0:64, 0:64])
                        nc.vector.tensor_copy(ctxs[64:128, :], ctxT[64:128, 64:128])
                        MTps = foldp.tile([C, C], F32, tag=f"mt{p}")
                        nc.tensor.matmul(
                            MTps[0:64, :], lhsT=ctxs[0:64, :],
                            rhs=wo_s[0:64, p, :],
                            start=True, stop=True, skip_group_check=True,
                        )
                        nc.tensor.matmul(
                            MTps[64:128, :], lhsT=ctxs[64:128, :],
                            rhs=wo_s[64:128, p, :],
                            start=True, stop=True, skip_group_check=True,
                        )
                        nc.vector.tensor_copy(MTp, MTps)

            # ---- pass 2: out = MT01^T @ eqnA01 + MT23^T @ eqnA23 + b ----
            with tc.tile_pool(name="finp", bufs=2, space="PSUM") as finp:
                osb = None
                for b2 in range(N // NB):
                    if b2 % (DMABLK // NB) == 0:
                        osb = osbp.tile([C, DMABLK], BF16, tag="osb")
                    nsl = slice(b2 * NB, (b2 + 1) * NB)
                    fin = finp.tile([C, NB], F32, tag="fin")
                    nc.tensor.matmul(
                        fin, lhsT=MT01, rhs=eqnA[:, 0, nsl],
                        start=True, stop=False, skip_group_check=True,
                    )
                    nc.tensor.matmul(
                        fin, lhsT=MT23, rhs=eqnA[:, 1, nsl],
                        start=False, stop=True, skip_group_check=True,
                    )
                    oi = b2 % (DMABLK // NB)
                    sub = osb[:, oi * NB : (oi + 1) * NB]
                    if b2 % 2 == 0:
                        nc.scalar.activation(
                            sub, fin, AF.Identity, bias=bo_s[:, 0:1], scale=1.0
                        )
                    else:
                        nc.vector.tensor_scalar_add(sub, fin, bo_s[:, 0:1])
                    if oi == (DMABLK // NB) - 1:
                        blk = b2 // (DMABLK // NB)
                        nc.sync.dma_start(
                            out=out[:, blk * DMABLK : (blk + 1) * DMABLK], in_=osb
                        )

    nc.compile()
    return nc


_NC_CACHE = None


def _host_inputs(x, w_qkv, w_out, b_out):
    bf = ml_dtypes.bfloat16
    b = x.shape[0]
    wqT = np.ascontiguousarray(np.asarray(w_qkv, dtype=np.float32).T.astype(bf))
    woT = np.ascontiguousarray(np.asarray(w_out, dtype=np.float32).T.astype(bf))
    bo = np.ascontiguousarray(np.asarray(b_out, dtype=np.float32).reshape(C, 1))
    xb = np.asarray(x, dtype=np.float32).reshape(b, C, N).astype(bf)
    in_maps = []
    for i in range(b):
        xi = np.ascontiguousarray(xb[i])
        # xt[p, t, c] = [x^T | 1][t*128+p, c]  (n on partitions, ones col appended)
        xta = np.empty((N, 129), dtype=bf)
        xta[:, 0:128] = xi.T
        xta[:, 128] = np.float32(1.0)
        xthost = np.ascontiguousarray(
            xta.reshape(NSUB, 128, 129).transpose(1, 0, 2)
        )
        in_maps.append({"x": xi, "xt": xthost, "wqT": wqT, "woT": woT, "bo": bo})
    return in_maps


def kernel(x, w_qkv, w_out, b_out):
    global _NC_CACHE
    if _NC_CACHE is None:
        _NC_CACHE = build_nc()
    nc = _NC_CACHE

    b = x.shape[0]
    in_maps = _host_inputs(x, w_qkv, w_out, b_out)
    res = run_bass_kernel_spmd(nc, in_maps, core_ids=list(range(b)))
    return np.stack(
        [
            np.asarray(res.results[i]["out"], dtype=np.float32).reshape(C, 128, 128)
            for i in range(b)
        ]
    )


# revision 8
# speedup vs baseline: 1.5447x; 1.0329x over previous
"""LinearAttention Trainium2 Bass kernel (v2).

kernel(**inputs) takes the full unsharded inputs from setup_inputs() and
returns the full output. Shards data-parallel over batch (b=8) across 8
NeuronCores; each core computes one batch item.

Per core, with n = h*w = 16384 pixels on SBUF partitions (128 px/tile):

  qk   = x^T @ w_qk           (512-col matmul per tile, bf16 PSUM)
  eqk  = exp(qk)              (one ACT instr per 4 tiles)
  sq   = sum_d exp(q)         (DVE reduce, per head)
  eqn  = exp(q) / sq          (GpSimd broadcast multiply by 1/sq)
  G_p  = sum_n ek_p [x^T | 1] (PE accumulation in PSUM over all tiles;
                               replaces materializing v: ctx = w_v @ G^T)
  eqnA = eqn^T                (PE transposes, staged to SBUF for pass 2)

Fold (once): ctx^T = w_v^T @ (G/s_k)^T ; MT_p = blockdiag(ctx)^T @ w_out^T
Pass 2: out = MT01^T @ eqnA01 + MT23^T @ eqnA23 + b_out   (bf16 out,
upcast to fp32 on host).
"""

import numpy as np
import ml_dtypes

import concourse.bass as bass
import concourse.tile as tile
from concourse import bacc, mybir
from concourse.bass_utils import run_bass_kernel_spmd
from concourse.masks import make_identity

F32 = mybir.dt.float32
BF16 = mybir.dt.bfloat16
AF = mybir.ActivationFunctionType

C = 128
N = 16384
HEADS = 4
DH = 64
INNER = HEADS * DH          # 256
QKV = 3 * INNER             # 768
NSUB = N // 128             # 128 tiles of 128 px
GS = 4                      # sub-tiles per group (batched elementwise)
NG = NSUB // GS             # 32 groups
DMABLK = 2048               # px per DMA transfer
NBLK = N // DMABLK          # 8 DMA blocks
NB = 512                    # pass-2 block (px)


def build_nc():
    nc = bacc.Bacc("TRN2", target_bir_lowering=False, debug=False, num_devices=8)

    x = nc.dram_tensor("x", [C, N], BF16, kind="ExternalInput")
    xt = nc.dram_tensor("xt", [C, NSUB, 129], BF16, kind="ExternalInput")
    wqT = nc.dram_tensor("wqT", [C, QKV], BF16, kind="ExternalInput")
    woT = nc.dram_tensor("woT", [INNER, C], BF16, kind="ExternalInput")
    bo = nc.dram_tensor("bo", [C, 1], F32, kind="ExternalInput")
    out = nc.dram_tensor("out", [C, N], BF16, kind="ExternalOutput")

    with tile.TileContext(nc) as tc:
        with (
            tc.tile_pool(name="consts", bufs=1) as consts,
            tc.tile_pool(name="eqa", bufs=1) as eqa,
            tc.tile_pool(name="xin", bufs=2) as xin,
            tc.tile_pool(name="xtin", bufs=2) as xtin,
            tc.tile_pool(name="work", bufs=2) as work,
            tc.tile_pool(name="small", bufs=2) as small,
            tc.tile_pool(name="folds", bufs=1) as folds,
            tc.tile_pool(name="osbp", bufs=2) as osbp,
        ):
            wq_s = consts.tile([C, QKV], BF16)
            nc.sync.dma_start(out=wq_s, in_=wqT[:, :])
            wo_s = consts.tile([C, 2, C], BF16)
            nc.sync.dma_start(out=wo_s[:, 0, :], in_=woT[0:128, :])
            nc.sync.dma_start(out=wo_s[:, 1, :], in_=woT[128:256, :])
            bo_s = consts.tile([C, 1], F32)
            nc.sync.dma_start(out=bo_s, in_=bo[:, :])
            ident = consts.tile([C, C], BF16)
            make_identity(nc, ident)

            # transposed, normalized exp(q): [:, 0, :] = heads 0/1, [:, 1, :] = 2/3
            eqnA = eqa.tile([C, 2, N], BF16)
            MT01 = consts.tile([C, C], BF16)
            MT23 = consts.tile([C, C], BF16)

            with tc.tile_pool(name="gp", bufs=1, space="PSUM") as gp:
                Gpk = gp.tile([C, 2, 132], F32)
                G01 = Gpk[:, 0, 0:129]
                G23 = Gpk[:, 1, 0:129]

                with (
                    tc.tile_pool(name="qkp", bufs=2, space="PSUM") as qkp,
                    tc.tile_pool(name="trp", bufs=3, space="PSUM") as trp,
                ):
                    x_blk = None
                    xt_blk = None
                    pending = []
                    for g in range(NG):
                        if g % (DMABLK // (GS * 128)) == 0:
                            blk = g // (DMABLK // (GS * 128))
                            x_blk = xin.tile([C, DMABLK], BF16, tag="x_blk")
                            nc.sync.dma_start(
                                out=x_blk,
                                in_=x[:, blk * DMABLK : (blk + 1) * DMABLK],
                            )
                            xt_blk = xtin.tile([C, 16, 129], BF16, tag="xt_blk")
                            nc.sync.dma_start(
                                out=xt_blk, in_=xt[:, blk * 16 : (blk + 1) * 16, :]
                            )

                        # exp over q|k: heads 0..3 = q, 4..7 = k
                        eqk = work.tile([C, GS, 8, DH], BF16, tag="eqk")
                        for h in range(GS // 2):
                            qk = qkp.tile([C, 2, 8, DH], F32, tag="qk")
                            for s2 in range(2):
                                s = h * 2 + s2
                                ti = (g * GS + s) % 16
                                xs = x_blk[:, ti * 128 : (ti + 1) * 128]
                                nc.tensor.matmul(
                                    qk[:, s2, :, :], lhsT=xs, rhs=wq_s[:, 0:512],
                                    start=True, stop=True, skip_group_check=True,
                                )
                            nc.scalar.activation(
                                eqk[:, 2 * h : 2 * h + 2, :, :], qk, AF.Exp
                            )

                        sq = small.tile([C, GS, HEADS, 1], F32, tag="sq")
                        nc.vector.reduce_sum(
                            sq, eqk[:, :, 0:4, :], axis=mybir.AxisListType.X
                        )
                        rq = small.tile([C, GS, HEADS, 1], F32, tag="rq")
                        nc.vector.reciprocal(rq, sq)
                        # deferred eqnA copy (2 groups back) keeps the DVE
                        # queue from gating the recip->mult chain
                        if pending and pending[0][1] <= g - 2:
                            tr_old, g_old = pending.pop(0)
                            nc.vector.tensor_copy(
                                eqnA[
                                    :, :,
                                    g_old * (GS * 128) : (g_old + 1) * (GS * 128),
                                ],
                                tr_old,
                            )
                        eqn = work.tile([C, GS, HEADS, DH], BF16, tag="eqn")
                        nc.gpsimd.tensor_mul(
                            eqn, eqk[:, :, 0:4, :],
                            rq.broadcast_to([C, GS, HEADS, DH]),
                        )

                        tr2 = trp.tile([C, 2, GS * 128], BF16, tag="tr2")
                        for s in range(GS):
                            t = g * GS + s
                            ti = t % 16
                            nc.tensor.matmul(
                                G01, lhsT=eqk[:, s, 4:6, :],
                                rhs=xt_blk[:, ti, :],
                                start=(t == 0), stop=(t == NSUB - 1),
                                skip_group_check=True,
                            )
                            nc.tensor.matmul(
                                G23, lhsT=eqk[:, s, 6:8, :],
                                rhs=xt_blk[:, ti, :],
                                start=(t == 0), stop=(t == NSUB - 1),
                                skip_group_check=True,
                            )
                            nc.tensor.transpose(
                                tr2[:, 0, s * 128 : (s + 1) * 128],
                                eqn[:, s, 0:2, :], ident,
                            )
                            nc.tensor.transpose(
                                tr2[:, 1, s * 128 : (s + 1) * 128],
                                eqn[:, s, 2:4, :], ident,
                            )
                        pending.append((tr2, g))
                    for tr_old, g_old in pending:
                        nc.vector.tensor_copy(
                            eqnA[
                                :, :, g_old * (GS * 128) : (g_old + 1) * (GS * 128)
                            ],
                            tr_old,
                        )

                # ---- fold: MT_p = blockdiag(ctx_p / s_k)^T @ w_out_p^T ----
                with tc.tile_pool(name="foldp", bufs=1, space="PSUM") as foldp:
                    for p, Gp, MTp in ((0, G01, MT01), (1, G23, MT23)):
                        rv = folds.tile([C, 1], F32, tag=f"rv{p}")
                        nc.vector.reciprocal(rv, Gp[:, 128:129])
                        Gn = folds.tile([C, C], BF16, tag=f"gn{p}")
                        nc.vector.tensor_scalar_mul(Gn, Gp[:, 0:128], rv[:, 0:1])
                        GTps = foldp.tile([C, C], BF16, tag=f"gt{p}")
                        nc.tensor.transpose(GTps, Gn, ident)
                        GTs = folds.tile([C, C], BF16, tag=f"gts{p}")
                        nc.vector.tensor_copy(GTs, GTps)
                        ctxT = foldp.tile([C, C], F32, tag=f"ctx{p}")
                        nc.tensor.matmul(
                            ctxT, lhsT=wq_s[:, 512 + 128 * p : 512 + 128 * (p + 1)],
                            rhs=GTs, start=True, stop=True, skip_group_check=True,
                        )
                        ctxs = folds.tile([C, 64], BF16, tag=f"cs{p}")
                        nc.vector.tensor_copy(ctxs[0:64, :], ctxT[0:64, 0:64])
                        nc.vector.tensor_copy(ctxs[64:128, :], ctxT[64:128, 64:128])
                        MTps = foldp.tile([C, C], F32, tag=f"mt{p}")
                        nc.tensor.matmul(
                            MTps[0:64, :], lhsT=ctxs[0:64, :],
                            rhs=wo_s[0:64, p, :],
                            start=True, stop=True, skip_group_check=True,
                        )
                        nc.tensor.matmul(
                            MTps[64:128, :], lhsT=ctxs[64:128, :],
                            rhs=wo_s[64:128, p, :],
                            start=True, stop=True, skip_group_check=True,
                        )
                        nc.vector.tensor_copy(MTp, MTps)

            # ---- pass 2: out = MT01^T @ eqnA01 + MT23^T @ eqnA23 + b ----
            with tc.tile_pool(name="finp", bufs=4, space="PSUM") as finp:
                osb = None
                for b2 in range(N // NB):
                    if b2 % (DMABLK // NB) == 0:
                        osb = osbp.tile([C, DMABLK], BF16, tag="osb")
                    nsl = slice(b2 * NB, (b2 + 1) * NB)
                    fin = finp.tile([C, NB], F32, tag="fin")
                    nc.tensor.matmul(
                        fin, lhsT=MT01, rhs=eqnA[:, 0, nsl],
                        start=True, stop=False, skip_group_check=True,
                    )
                    nc.tensor.matmul(
                        fin, lhsT=MT23, rhs=eqnA[:, 1, nsl],
                        start=False, stop=True, skip_group_check=True,
                    )
                    oi = b2 % (DMABLK // NB)
                    sub = osb[:, oi * NB : (oi + 1) * NB]
                    if b2 % 2 == 0:
                        nc.scalar.activation(
                            sub, fin, AF.Identity, bias=bo_s[:, 0:1], scale=1.0
                        )
                    else:
                        nc.vector.tensor_scalar_add(sub, fin, bo_s[:, 0:1])
                    if oi == (DMABLK // NB) - 1:
                        blk = b2 // (DMABLK // NB)
                        nc.sync.dma_start(
                            out=out[:, blk * DMABLK : (blk + 1) * DMABLK], in_=osb
                        )

    nc.compile()
    return nc


_NC_CACHE = None


def _host_inputs(x, w_qkv, w_out, b_out):
    bf = ml_dtypes.bfloat16
    b = x.shape[0]
    wqT = np.ascontiguousarray(np.asarray(w_qkv, dtype=np.float32).T.astype(bf))
    woT = np.ascontiguousarray(np.asarray(w_out, dtype=np.float32).T.astype(bf))
    bo = np.ascontiguousarray(np.asarray(b_out, dtype=np.float32).reshape(C, 1))
    xb = np.asarray(x, dtype=np.float32).reshape(b, C, N).astype(bf)
    in_maps = []
    for i in range(b):
        xi = np.ascontiguousarray(xb[i])
        # xt[p, t, c] = [x^T | 1][t*128+p, c]  (n on partitions, ones col appended)
        xta = np.empty((N, 129), dtype=bf)
        xta[:, 0:128] = xi.T
        xta[:, 128] = np.float32(1.0)
        xthost = np.ascontiguousarray(
            xta.reshape(NSUB, 128, 129).transpose(1, 0, 2)
        )
        in_maps.append({"x": xi, "xt": xthost, "wqT": wqT, "woT": woT, "bo": bo})
    return in_maps


def kernel(x, w_qkv, w_out, b_out):
    global _NC_CACHE
    if _NC_CACHE is None:
        _NC_CACHE = build_nc()
    nc = _NC_CACHE

    b = x.shape[0]
    in_maps = _host_inputs(x, w_qkv, w_out, b_out)
    res = run_bass_kernel_spmd(nc, in_maps, core_ids=list(range(b)))
    return np.stack(
        [
            np.asarray(res.results[i]["out"], dtype=np.float32).reshape(C, 128, 128)
            for i in range(b)
        ]
    )


# revision 10
# speedup vs baseline: 1.7008x; 1.1010x over previous
"""LinearAttention Trainium2 Bass kernel (v2).

kernel(**inputs) takes the full unsharded inputs from setup_inputs() and
returns the full output. Shards data-parallel over batch (b=8) across 8
NeuronCores; each core computes one batch item.

Per core, with n = h*w = 16384 pixels on SBUF partitions (128 px/tile):

  qk   = x^T @ w_qk           (512-col matmul per tile, bf16 PSUM)
  eqk  = exp(qk)              (one ACT instr per 4 tiles)
  sq   = sum_d exp(q)         (DVE reduce, per head)
  eqn  = exp(q) / sq          (GpSimd broadcast multiply by 1/sq)
  G_p  = sum_n ek_p [x^T | 1] (PE accumulation in PSUM over all tiles;
                               replaces materializing v: ctx = w_v @ G^T)
  eqnA = eqn^T                (PE transposes, staged to SBUF for pass 2)

Fold (once): ctx^T = w_v^T @ (G/s_k)^T ; MT_p = blockdiag(ctx)^T @ w_out^T
Pass 2: out = MT01^T @ eqnA01 + MT23^T @ eqnA23 + b_out   (bf16 out,
upcast to fp32 on host).
"""

import numpy as np
import ml_dtypes

import concourse.bass as bass
import concourse.tile as tile
from concourse import bacc, mybir
from concourse.bass_utils import run_bass_kernel_spmd
from concourse.masks import make_identity

F32 = mybir.dt.float32
BF16 = mybir.dt.bfloat16
AF = mybir.ActivationFunctionType

C = 128
N = 16384
HEADS = 4
DH = 64
INNER = HEADS * DH          # 256
QKV = 3 * INNER             # 768
NSUB = N // 128             # 128 tiles of 128 px
GS = 4                      # sub-tiles per group (batched elementwise)
NG = NSUB // GS             # 32 groups
DMABLK = 2048               # px per DMA transfer
NBLK = N // DMABLK          # 8 DMA blocks
NB = 512                    # pass-2 block (px)


def build_nc():
    nc = bacc.Bacc("TRN2", target_bir_lowering=False, debug=False, num_devices=8)

    x = nc.dram_tensor("x", [C, N], BF16, kind="ExternalInput")
    xt = nc.dram_tensor("xt", [C, NSUB, 129], BF16, kind="ExternalInput")
    wqT = nc.dram_tensor("wqT", [C, QKV], BF16, kind="ExternalInput")
    woT = nc.dram_tensor("woT", [INNER, C], BF16, kind="ExternalInput")
    bo = nc.dram_tensor("bo", [C, 1], F32, kind="ExternalInput")
    out = nc.dram_tensor("out", [C, N], BF16, kind="ExternalOutput")

    with tile.TileContext(nc) as tc:
        with (
            tc.tile_pool(name="consts", bufs=1) as consts,
            tc.tile_pool(name="eqa", bufs=1) as eqa,
            tc.tile_pool(name="xin", bufs=2) as xin,
            tc.tile_pool(name="xtin", bufs=2) as xtin,
            tc.tile_pool(name="work", bufs=3) as work,
            tc.tile_pool(name="small", bufs=4) as small,
            tc.tile_pool(name="folds", bufs=1) as folds,
            tc.tile_pool(name="osbp", bufs=2) as osbp,
        ):
            wq_s = consts.tile([C, QKV], BF16)
            nc.sync.dma_start(out=wq_s, in_=wqT[:, :])
            wo_s = consts.tile([C, 2, C], BF16)
            nc.sync.dma_start(out=wo_s[:, 0, :], in_=woT[0:128, :])
            nc.sync.dma_start(out=wo_s[:, 1, :], in_=woT[128:256, :])
            bo_s = consts.tile([C, 1], F32)
            nc.sync.dma_start(out=bo_s, in_=bo[:, :])
            ident = consts.tile([C, C], BF16)
            make_identity(nc, ident)

            # transposed, normalized exp(q): [:, 0, :] = heads 0/1, [:, 1, :] = 2/3
            eqnA = eqa.tile([C, 2, N], BF16)
            MT01 = consts.tile([C, C], BF16)
            MT23 = consts.tile([C, C], BF16)

            with tc.tile_pool(name="gp", bufs=1, space="PSUM") as gp:
                G01 = gp.tile([C, 129], F32)
                G23 = gp.tile([C, 129], F32)

                with (
                    tc.tile_pool(name="qkp", bufs=2, space="PSUM") as qkp,
                    tc.tile_pool(name="trp", bufs=2, space="PSUM") as trp,
                ):
                    x_blk = None
                    xt_blk = None
                    pending = []
                    for g in range(NG):
                        if g % (DMABLK // (GS * 128)) == 0:
                            blk = g // (DMABLK // (GS * 128))
                            x_blk = xin.tile([C, DMABLK], BF16, tag="x_blk")
                            nc.sync.dma_start(
                                out=x_blk,
                                in_=x[:, blk * DMABLK : (blk + 1) * DMABLK],
                            )
                            xt_blk = xtin.tile([C, 16, 129], BF16, tag="xt_blk")
                            nc.sync.dma_start(
                                out=xt_blk, in_=xt[:, blk * 16 : (blk + 1) * 16, :]
                            )

                        # exp over q|k: heads 0..3 = q, 4..7 = k
                        eqk = work.tile([C, GS, 8, DH], BF16, tag="eqk")
                        for h in range(GS // 2):
                            qk = qkp.tile([C, 2, 8, DH], F32, tag="qk")
                            for s2 in range(2):
                                s = h * 2 + s2
                                ti = (g * GS + s) % 16
                                xs = x_blk[:, ti * 128 : (ti + 1) * 128]
                                nc.tensor.matmul(
                                    qk[:, s2, :, :], lhsT=xs, rhs=wq_s[:, 0:512],
                                    start=True, stop=True, skip_group_check=True,
                                )
                            nc.scalar.activation(
                                eqk[:, 2 * h : 2 * h + 2, :, :], qk, AF.Exp
                            )

                        sq = small.tile([C, GS, HEADS, 1], F32, tag="sq")
                        nc.vector.reduce_sum(
                            sq, eqk[:, :, 0:4, :], axis=mybir.AxisListType.X
                        )
                        rq = small.tile([C, GS, HEADS, 1], F32, tag="rq")
                        nc.vector.reciprocal(rq, sq)
                        # deferred eqnA copy (2 groups back) keeps the DVE
                        # queue from gating the recip->mult chain
                        if pending and pending[0][1] <= g - 2:
                            tr_old, g_old = pending.pop(0)
                            nc.vector.tensor_copy(
                                eqnA[
                                    :, :,
                                    g_old * (GS * 128) : (g_old + 1) * (GS * 128),
                                ],
                                tr_old,
                            )
                        eqn = work.tile([C, GS, HEADS, DH], BF16, tag="eqn")
                        nc.gpsimd.tensor_mul(
                            eqn, eqk[:, :, 0:4, :],
                            rq.broadcast_to([C, GS, HEADS, DH]),
                        )

                        tr2 = trp.tile([C, 2, GS * 128], BF16, tag="tr2")
                        for s in range(GS):
                            t = g * GS + s
                            ti = t % 16
                            nc.tensor.matmul(
                                G01, lhsT=eqk[:, s, 4:6, :],
                                rhs=xt_blk[:, ti, :],
                                start=(t == 0), stop=(t == NSUB - 1),
                                skip_group_check=True,
                            )
                            nc.tensor.matmul(
                                G23, lhsT=eqk[:, s, 6:8, :],
                                rhs=xt_blk[:, ti, :],
                                start=(t == 0), stop=(t == NSUB - 1),
                                skip_group_check=True,
                            )
                            nc.tensor.transpose(
                                tr2[:, 0, s * 128 : (s + 1) * 128],
                                eqn[:, s, 0:2, :], ident,
                            )
                            nc.tensor.transpose(
                                tr2[:, 1, s * 128 : (s + 1) * 128],
                                eqn[:, s, 2:4, :], ident,
                            )
                        pending.append((tr2, g))
                    for tr_old, g_old in pending:
                        nc.vector.tensor_copy(
                            eqnA[
                                :, :, g_old * (GS * 128) : (g_old + 1) * (GS * 128)
                            ],
                            tr_old,
                        )

                # ---- fold: MT_p = blockdiag(ctx_p / s_k)^T @ w_out_p^T ----
                with tc.tile_pool(name="foldp", bufs=1, space="PSUM") as foldp:
                    for p, Gp, MTp in ((0, G01, MT01), (1, G23, MT23)):
                        rv = folds.tile([C, 1], F32, tag=f"rv{p}")
                        nc.vector.reciprocal(rv, Gp[:, 128:129])
                        Gn = folds.tile([C, C], BF16, tag=f"gn{p}")
                        nc.vector.tensor_scalar_mul(Gn, Gp[:, 0:128], rv[:, 0:1])
                        GTps = foldp.tile([C, C], BF16, tag=f"gt{p}")
                        nc.tensor.transpose(GTps, Gn, ident)
                        GTs = folds.tile([C, C], BF16, tag=f"gts{p}")
                        nc.vector.tensor_copy(GTs, GTps)
                        ctxT = foldp.tile([C, C], F32, tag=f"ctx{p}")
                        nc.tensor.matmul(
                            ctxT, lhsT=wq_s[:, 512 + 128 * p : 512 + 128 * (p + 1)],
                            rhs=GTs, start=True, stop=True, skip_group_check=True,
                        )
                        ctxs = folds.tile([C, 64], BF16, tag=f"cs{p}")
                        nc.vector.tensor_copy(ctxs[0:64, :], ctxT[0:64, 0:64])
                        nc.vector.tensor_copy(ctxs[64:128, :], ctxT[64:128, 64:128])
                        MTps = foldp.tile([C, C], F32, tag=f"mt{p}")
                        nc.tensor.matmul(
                            MTps[0:64, :], lhsT=ctxs[0:64, :],
                            rhs=wo_s[0:64, p, :],
                            start=True, stop=True, skip_group_check=True,
                        )
                        nc.tensor.matmul(
                            MTps[64:128, :], lhsT=ctxs[64:128, :],
                            rhs=wo_s[64:128, p, :],
                            start=True, stop=True, skip_group_check=True,
                        )
                        nc.vector.tensor_copy(MTp, MTps)

            # ---- pass 2: out = MT01^T @ eqnA01 + MT23^T @ eqnA23 + b ----
            with tc.tile_pool(name="finp", bufs=4, space="PSUM") as finp:
                osb = None
                for b2 in range(N // NB):
                    if b2 % (DMABLK // NB) == 0:
                        osb = osbp.tile([C, DMABLK], BF16, tag="osb")
                    nsl = slice(b2 * NB, (b2 + 1) * NB)
                    fin = finp.tile([C, NB], F32, tag="fin")
                    nc.tensor.matmul(
                        fin, lhsT=MT01, rhs=eqnA[:, 0, nsl],
                        start=True, stop=False, skip_group_check=True,
                    )
                    nc.tensor.matmul(
                        fin, lhsT=MT23, rhs=eqnA[:, 1, nsl],
                        start=False, stop=True, skip_group_check=True,
                    )
                    oi = b2 % (DMABLK // NB)
                    sub = osb[:, oi * NB : (oi + 1) * NB]
                    if b2 % 2 == 0:
                        nc.scalar.activation(
                            sub, fin, AF.Identity, bias=bo_s[:, 0:1], scale=1.0
                        )
                    else:
                        nc.vector.tensor_scalar_add(sub, fin, bo_s[:, 0:1])
                    if oi == (DMABLK // NB) - 1:
                        blk = b2 // (DMABLK // NB)
                        nc.sync.dma_start(
                            out=out[:, blk * DMABLK : (blk + 1) * DMABLK], in_=osb
                        )

    nc.compile()
    return nc


_NC_CACHE = None


def _host_inputs(x, w_qkv, w_out, b_out):
    bf = ml_dtypes.bfloat16
    b = x.shape[0]
    wqT = np.ascontiguousarray(np.asarray(w_qkv, dtype=np.float32).T.astype(bf))
    woT = np.ascontiguousarray(np.asarray(w_out, dtype=np.float32).T.astype(bf))
    bo = np.ascontiguousarray(np.asarray(b_out, dtype=np.float32).reshape(C, 1))
    xb = np.asarray(x, dtype=np.float32).reshape(b, C, N).astype(bf)
    in_maps = []
    for i in range(b):
        xi = np.ascontiguousarray(xb[i])
        # xt[p, t, c] = [x^T | 1][t*128+p, c]  (n on partitions, ones col appended)
        xta = np.empty((N, 129), dtype=bf)
        xta[:, 0:128] = xi.T
        xta[:, 128] = np.float32(1.0)
        xthost = np.ascontiguousarray(
            xta.reshape(NSUB, 128, 129).transpose(1, 0, 2)
        )
        in_maps.append({"x": xi, "xt": xthost, "wqT": wqT, "woT": woT, "bo": bo})
    return in_maps


def kernel(x, w_qkv, w_out, b_out):
    global _NC_CACHE
    if _NC_CACHE is None:
        _NC_CACHE = build_nc()
    nc = _NC_CACHE

    b = x.shape[0]
    in_maps = _host_inputs(x, w_qkv, w_out, b_out)
    res = run_bass_kernel_spmd(nc, in_maps, core_ids=list(range(b)))
    return np.stack(
        [
            np.asarray(res.results[i]["out"], dtype=np.float32).reshape(C, 128, 128)
            for i in range(b)
        ]
    )


# revision 18
# speedup vs baseline: 1.7359x; 1.0206x over previous
"""LinearAttention Trainium2 Bass kernel (v2).

kernel(**inputs) takes the full unsharded inputs from setup_inputs() and
returns the full output. Shards data-parallel over batch (b=8) across 8
NeuronCores; each core computes one batch item.

Per core, with n = h*w = 16384 pixels on SBUF partitions (128 px/tile):

  qk   = x^T @ w_qk           (512-col matmul per tile, bf16 PSUM)
  eqk  = exp(qk)              (one ACT instr per 4 tiles)
  sq   = sum_d exp(q)         (DVE reduce, per head)
  eqn  = exp(q) / sq          (GpSimd broadcast multiply by 1/sq)
  G_p  = sum_n ek_p [x^T | 1] (PE accumulation in PSUM over all tiles;
                               replaces materializing v: ctx = w_v @ G^T)
  eqnA = eqn^T                (PE transposes, staged to SBUF for pass 2)

Fold (once): ctx^T = w_v^T @ (G/s_k)^T ; MT_p = blockdiag(ctx)^T @ w_out^T
Pass 2: out = MT01^T @ eqnA01 + MT23^T @ eqnA23 + b_out   (bf16 out,
upcast to fp32 on host).
"""

import numpy as np
import ml_dtypes

import concourse.bass as bass
import concourse.tile as tile
from concourse import bacc, mybir
from concourse.bass_utils import run_bass_kernel_spmd
from concourse.masks import make_identity

F32 = mybir.dt.float32
BF16 = mybir.dt.bfloat16
AF = mybir.ActivationFunctionType

C = 128
N = 16384
HEADS = 4
DH = 64
INNER = HEADS * DH          # 256
QKV = 3 * INNER             # 768
NSUB = N // 128             # 128 tiles of 128 px
GS = 4                      # sub-tiles per group (batched elementwise)
NG = NSUB // GS             # 32 groups
DMABLK = 2048               # px per DMA transfer
NBLK = N // DMABLK          # 8 DMA blocks
NB = 512                    # pass-2 block (px)


def build_nc():
    nc = bacc.Bacc("TRN2", target_bir_lowering=False, debug=False, num_devices=8)

    x = nc.dram_tensor("x", [C, N], BF16, kind="ExternalInput")
    xt = nc.dram_tensor("xt", [C, NSUB, 129], BF16, kind="ExternalInput")
    wqT = nc.dram_tensor("wqT", [C, QKV], BF16, kind="ExternalInput")
    woT = nc.dram_tensor("woT", [INNER, C], BF16, kind="ExternalInput")
    bo = nc.dram_tensor("bo", [C, 1], F32, kind="ExternalInput")
    out = nc.dram_tensor("out", [C, N], BF16, kind="ExternalOutput")

    with tile.TileContext(nc) as tc:
        with (
            tc.tile_pool(name="consts", bufs=1) as consts,
            tc.tile_pool(name="eqa", bufs=1) as eqa,
            tc.tile_pool(name="xin", bufs=2) as xin,
            tc.tile_pool(name="xtin", bufs=2) as xtin,
            tc.tile_pool(name="work", bufs=3) as work,
            tc.tile_pool(name="small", bufs=4) as small,
            tc.tile_pool(name="folds", bufs=1) as folds,
            tc.tile_pool(name="osbp", bufs=3) as osbp,
        ):
            wq_s = consts.tile([C, QKV], BF16)
            nc.sync.dma_start(out=wq_s, in_=wqT[:, :])
            wo_s = consts.tile([C, 2, C], BF16)
            nc.sync.dma_start(out=wo_s[:, 0, :], in_=woT[0:128, :])
            nc.sync.dma_start(out=wo_s[:, 1, :], in_=woT[128:256, :])
            bo_s = consts.tile([C, 1], F32)
            nc.sync.dma_start(out=bo_s, in_=bo[:, :])
            ident = consts.tile([C, C], BF16)
            make_identity(nc, ident)


            # transposed, normalized exp(q): [:, 0, :] = heads 0/1, [:, 1, :] = 2/3
            eqnA = eqa.tile([C, 2, N], BF16)
            MT01 = consts.tile([C, C], BF16)
            MT23 = consts.tile([C, C], BF16)

            with tc.tile_pool(name="gp", bufs=1, space="PSUM") as gp:
                # G01/G23 share one PSUM bank: zero it once and accumulate
                # with start=False throughout (start=True resets the whole
                # bank, which would wipe the sibling's contribution)
                Gpk = gp.tile([C, 2, 132], F32)
                G01 = Gpk[:, 0, 0:129]
                G23 = Gpk[:, 1, 0:129]
                nc.vector.memzero(Gpk)

                with (
                    tc.tile_pool(name="qkp", bufs=2, space="PSUM") as qkp,
                    tc.tile_pool(name="trp", bufs=3, space="PSUM") as trp,
                ):
                    x_blk = None
                    xt_blk = None
                    pending = []
                    for g in range(NG):
                        if g % (DMABLK // (GS * 128)) == 0:
                            blk = g // (DMABLK // (GS * 128))
                            x_blk = xin.tile([C, DMABLK], BF16, tag="x_blk")
                            nc.sync.dma_start(
                                out=x_blk,
                                in_=x[:, blk * DMABLK : (blk + 1) * DMABLK],
                            )
                            xt_blk = xtin.tile([C, 16, 129], BF16, tag="xt_blk")
                            nc.sync.dma_start(
                                out=xt_blk, in_=xt[:, blk * 16 : (blk + 1) * 16, :]
                            )

                        # exp over q|k: heads 0..3 = q, 4..7 = k
                        eqk = work.tile([C, GS, 8, DH], BF16, tag="eqk")
                        for h in range(GS // 2):
                            qk = qkp.tile([C, 2, 8, DH], F32, tag="qk")
                            for s2 in range(2):
                                s = h * 2 + s2
                                ti = (g * GS + s) % 16
                                xs = x_blk[:, ti * 128 : (ti + 1) * 128]
                                nc.tensor.matmul(
                                    qk[:, s2, :, :], lhsT=xs, rhs=wq_s[:, 0:512],
                                    start=True, stop=True, skip_group_check=True,
                                )
                            nc.scalar.activation(
                                eqk[:, 2 * h : 2 * h + 2, :, :], qk, AF.Exp
                            )

                        sq = small.tile([C, GS, HEADS, 1], F32, tag="sq")
                        nc.vector.reduce_sum(
                            sq, eqk[:, :, 0:4, :], axis=mybir.AxisListType.X
                        )
                        rq = small.tile([C, GS, HEADS, 1], F32, tag="rq")
                        nc.vector.reciprocal(rq, sq)
                        # deferred eqnA copy (2 groups back) keeps the DVE
                        # queue from gating the reduce chain
                        if pending and pending[0][1] <= g - 2:
                            tr_old, g_old = pending.pop(0)
                            nc.vector.tensor_copy(
                                eqnA[
                                    :, :,
                                    g_old * (GS * 128) : (g_old + 1) * (GS * 128),
                                ],
                                tr_old,
                            )
                        eqn = work.tile([C, GS, HEADS, DH], BF16, tag="eqn")
                        nc.gpsimd.tensor_mul(
                            eqn, eqk[:, :, 0:4, :],
                            rq.broadcast_to([C, GS, HEADS, DH]),
                        )

                        tr2 = trp.tile([C, 2, GS * 128], BF16, tag="tr2")
                        for s in range(GS):
                            t = g * GS + s
                            ti = t % 16
                            nc.tensor.matmul(
                                G01, lhsT=eqk[:, s, 4:6, :],
                                rhs=xt_blk[:, ti, :],
                                start=False, stop=(t == NSUB - 1),
                                skip_group_check=True,
                            )
                            nc.tensor.matmul(
                                G23, lhsT=eqk[:, s, 6:8, :],
                                rhs=xt_blk[:, ti, :],
                                start=False, stop=(t == NSUB - 1),
                                skip_group_check=True,
                            )
                            nc.tensor.transpose(
                                tr2[:, 0, s * 128 : (s + 1) * 128],
                                eqn[:, s, 0:2, :], ident,
                            )
                            nc.tensor.transpose(
                                tr2[:, 1, s * 128 : (s + 1) * 128],
                                eqn[:, s, 2:4, :], ident,
                            )
                        pending.append((tr2, g))
                    for tr_old, g_old in pending:
                        nc.vector.tensor_copy(
                            eqnA[
                                :, :, g_old * (GS * 128) : (g_old + 1) * (GS * 128)
                            ],
                            tr_old,
                        )

                # ---- fold: MT_p = blockdiag(ctx_p / s_k)^T @ w_out_p^T ----
                with tc.tile_pool(name="foldp", bufs=1, space="PSUM") as foldp:
                    for p, Gp, MTp in ((0, G01, MT01), (1, G23, MT23)):
                        rv = folds.tile([C, 1], F32, tag=f"rv{p}")
                        nc.vector.reciprocal(rv, Gp[:, 128:129])
                        Gn = folds.tile([C, C], BF16, tag=f"gn{p}")
                        nc.vector.tensor_scalar_mul(Gn, Gp[:, 0:128], rv[:, 0:1])
                        GTps = foldp.tile([C, C], BF16, tag=f"gt{p}")
                        nc.tensor.transpose(GTps, Gn, ident)
                        GTs = folds.tile([C, C], BF16, tag=f"gts{p}")
                        nc.vector.tensor_copy(GTs, GTps)
                        ctxT = foldp.tile([C, C], F32, tag=f"ctx{p}")
                        nc.tensor.matmul(
                            ctxT, lhsT=wq_s[:, 512 + 128 * p : 512 + 128 * (p + 1)],
                            rhs=GTs, start=True, stop=True, skip_group_check=True,
                        )
                        ctxs = folds.tile([C, 64], BF16, tag=f"cs{p}")
                        nc.vector.tensor_copy(ctxs[0:64, :], ctxT[0:64, 0:64])
                        nc.vector.tensor_copy(ctxs[64:128, :], ctxT[64:128, 64:128])
                        MTps = foldp.tile([C, C], F32, tag=f"mt{p}")
                        nc.tensor.matmul(
                            MTps[0:64, :], lhsT=ctxs[0:64, :],
                            rhs=wo_s[0:64, p, :],
                            start=True, stop=True, skip_group_check=True,
                        )
                        nc.tensor.matmul(
                            MTps[64:128, :], lhsT=ctxs[64:128, :],
                            rhs=wo_s[64:128, p, :],
                            start=True, stop=True, skip_group_check=True,
                        )
                        nc.vector.tensor_copy(MTp, MTps)

            # ---- pass 2: out = MT01^T @ eqnA01 + MT23^T @ eqnA23 + b ----
            with tc.tile_pool(name="finp", bufs=4, space="PSUM") as finp:
                OB = 1024
                osb = None
                for b2 in range(N // NB):
                    if b2 % (OB // NB) == 0:
                        osb = osbp.tile([C, OB], BF16, tag="osb")
                    nsl = slice(b2 * NB, (b2 + 1) * NB)
                    fin = finp.tile([C, NB], F32, tag="fin")
                    nc.tensor.matmul(
                        fin, lhsT=MT01, rhs=eqnA[:, 0, nsl],
                        start=True, stop=False, skip_group_check=True,
                    )
                    nc.tensor.matmul(
                        fin, lhsT=MT23, rhs=eqnA[:, 1, nsl],
                        start=False, stop=True, skip_group_check=True,
                    )
                    oi = b2 % (OB // NB)
                    sub = osb[:, oi * NB : (oi + 1) * NB]
                    if b2 % 2 == 0:
                        nc.scalar.activation(
                            sub, fin, AF.Identity, bias=bo_s[:, 0:1], scale=1.0
                        )
                    else:
                        nc.vector.tensor_scalar_add(sub, fin, bo_s[:, 0:1])
                    if oi == (OB // NB) - 1:
                        blk = b2 // (OB // NB)
                        nc.sync.dma_start(
                            out=out[:, blk * OB : (blk + 1) * OB], in_=osb
                        )

    nc.compile()
    return nc


_NC_CACHE = None


def _host_inputs(x, w_qkv, w_out, b_out):
    bf = ml_dtypes.bfloat16
    b = x.shape[0]
    wqT = np.ascontiguousarray(np.asarray(w_qkv, dtype=np.float32).T.astype(bf))
    woT = np.ascontiguousarray(np.asarray(w_out, dtype=np.float32).T.astype(bf))
    bo = np.ascontiguousarray(np.asarray(b_out, dtype=np.float32).reshape(C, 1))
    xb = np.asarray(x, dtype=np.float32).reshape(b, C, N).astype(bf)
    in_maps = []
    for i in range(b):
        xi = np.ascontiguousarray(xb[i])
        # xt[p, t, c] = [x^T | 1][t*128+p, c]  (n on partitions, ones col appended)
        xta = np.empty((N, 129), dtype=bf)
        xta[:, 0:128] = xi.T
        xta[:, 128] = np.float32(1.0)
        xthost = np.ascontiguousarray(
            xta.reshape(NSUB, 128, 129).transpose(1, 0, 2)
        )
        in_maps.append({"x": xi, "xt": xthost, "wqT": wqT, "woT": woT, "bo": bo})
    return in_maps


def kernel(x, w_qkv, w_out, b_out):
    global _NC_CACHE
    if _NC_CACHE is None:
        _NC_CACHE = build_nc()
    nc = _NC_CACHE

    b = x.shape[0]
    in_maps = _host_inputs(x, w_qkv, w_out, b_out)
    res = run_bass_kernel_spmd(nc, in_maps, core_ids=list(range(b)))
    return np.stack(
        [
            np.asarray(res.results[i]["out"], dtype=np.float32).reshape(C, 128, 128)
            for i in range(b)
        ]
    )
